# revision 1
# baseline (speedup 1.0000x reference)
"""Trainium2 Bass kernel for HadamardTernaryLinear.

y = reshape( (FHT_g(x*alpha) @grouped w_q) -> FHT_h -> *beta ), with
w_q = BitNet-style absmean ternary quantization of weight.

Strategy: data-parallel over the 8192 tokens across 8 NeuronCores (1024
tokens/core, no collectives). Per core, a 5-pass TensorEngine pipeline in
bf16 (Hadamard and ternary weight matrices are exact +-1/0 in bf16; the
quant scale, alpha and beta are folded into f32 host prep / the final
f32 PSUM drain):

  P1 stat-FHTg : MM(lhsT=x-tile, rhs=I4(x)H32)   -> token-major  [tok, (i,h)]
  P2 T-regroup : MM(lhsT=sel,    rhs=I)          -> group-major  [i, tok] per h
  P3 stat-GM   : MM(lhsT=xb,     rhs=wqT[h])     -> token-major  [tok, (h,o)]
  P4 T-regroup : MM(lhsT=sel,    rhs=I)          -> A-layout     [(o',h), tok]
  P5 mov-FHTh  : MM(lhsT=I4(x)H32, rhs=ypa)      -> [(o',g), tok] f32 PSUM
  drain P5 on ScalarE with per-row scale = beta*quantscale/32, store f32.

Feature order on device is i-major (f' = i*32 + g), prepared host-side so
DMA transposes see contiguous 128-column blocks.
"""

import functools
import sys

for _p in ("/opt/trn_rl_repo",):
    if _p not in sys.path:
        sys.path.insert(0, _p)

import ml_dtypes
import numpy as np

import concourse.mybir as mybir
import concourse.tile as tile
from concourse import bacc
from concourse.bass_utils import run_bass_kernel_spmd

G = 32
IO = 128  # in_o
OO = 128  # out_o
D = G * IO  # 4096
NCORES = 8
B, T = 4, 2048
BT = B * T
TOKC = BT // NCORES  # tokens per core
ST = 512  # supertile tokens
NST = TOKC // ST

DTB = mybir.dt.bfloat16
DTF = mybir.dt.float32
BF16 = ml_dtypes.bfloat16


def _hadamard(n):
    H = np.array([[1.0]], dtype=np.float32)
    while H.shape[0] < n:
        H = np.block([[H, H], [H, -H]])
    return H  # +-1, symmetric


def build_body(nc, tc, xin, hm, idm, wqm, bc, yout, loop_r=1):
    """Emit the per-core program. All APs are DRAM tensors."""
    CH = ST // 128  # 128-token chunks per supertile

    with (
        tc.tile_pool(name="const", bufs=1) as cpool,
        tc.tile_pool(name="stage", bufs=1) as spool,
        tc.tile_pool(name="xa", bufs=1) as xapool,
        tc.tile_pool(name="yf", bufs=6) as ypool,
        tc.tile_pool(name="psum", bufs=6, space="PSUM") as pspool,
    ):
        hmt = cpool.tile([128, 128], DTB, tag="hm")
        nc.sync.dma_start(hmt[:], hm[:])
        idt = cpool.tile([128, 128], DTB, tag="id")
        nc.sync.dma_start(idt[:], idm[:])
        wqt = cpool.tile([128, G * OO], DTB, tag="wq")
        nc.sync.dma_start(wqt[:], wqm[:])
        bct = cpool.tile([128, G], DTF, tag="bc")
        nc.sync.dma_start(bct[:], bc[:])

        def supertile(st):
            t0 = st * ST
            # ---- loads: DMA-xbar transpose -> A-layout tiles [(i',g), tok]
            xa = []
            for k in range(32):
                xk = xapool.tile([128, ST], DTB, tag=f"xa{k}")
                nc.sync.dma_start_transpose(
                    xk[:], xin[t0 : t0 + ST, k * 128 : (k + 1) * 128]
                )
                xa.append(xk)

            # ---- P1: stationary FHT_g -> token-major tm_c [tok, i*32+h]
            tms = []
            for c in range(CH):
                tm = spool.tile([128, D], DTB, tag=f"tm{c}")
                for kq in range(8):
                    ps = pspool.tile([128, 512], DTF, tag="ps")
                    for kk in range(4):
                        k = kq * 4 + kk
                        nc.tensor.matmul(
                            ps[:, kk * 128 : (kk + 1) * 128],
                            lhsT=xa[k][:, c * 128 : (c + 1) * 128],
                            rhs=hmt[:],
                            start=True,
                            stop=True,
                        )
                    nc.vector.tensor_copy(tm[:, kq * 512 : (kq + 1) * 512], ps[:])
                tms.append(tm)

            # ---- P2: transpose-regroup -> xb_h [i, tok] (group-major)
            xb = []
            for h in range(32):
                ps = pspool.tile([128, ST], DTF, tag="ps")
                for c in range(CH):
                    sel = tms[c].rearrange("p (i h) -> p h i", h=32)[:, h, :]
                    nc.tensor.matmul(
                        ps[:, c * 128 : (c + 1) * 128],
                        lhsT=sel,
                        rhs=idt[:],
                        start=True,
                        stop=True,
                    )
                xbh = spool.tile([128, ST], DTB, tag=f"xb{h}")
                nc.scalar.copy(xbh[:], ps[:])
                xb.append(xbh)

            # ---- P3: stationary grouped matmul -> token-major tm2_c.
            # Drain scatters to o-major column order (col = o*32 + h) so P4's
            # weight selection is a contiguous 128-column slice (walrus allows
            # only 2D APs on the matmul stationary operand).
            tm2s = []
            for c in range(CH):
                tm2 = spool.tile([128, D], DTB, tag=f"tm2_{c}")
                tm2v = tm2.rearrange("p (o h) -> p h o", h=32)
                for hq in range(8):
                    ps = pspool.tile([128, 512], DTF, tag="ps")
                    for hh in range(4):
                        h = hq * 4 + hh
                        nc.tensor.matmul(
                            ps[:, hh * 128 : (hh + 1) * 128],
                            lhsT=xb[h][:, c * 128 : (c + 1) * 128],
                            rhs=wqt[:, h * 128 : (h + 1) * 128],
                            start=True,
                            stop=True,
                        )
                    nc.vector.tensor_copy(tm2v[:, hq * 4 : (hq + 1) * 4, :], ps[:])
                tm2s.append(tm2)

            # ---- P4: transpose-regroup -> ypa_m [(o',h), tok]
            ypa = []
            for m in range(32):
                ps = pspool.tile([128, ST], DTF, tag="ps")
                for c in range(CH):
                    nc.tensor.matmul(
                        ps[:, c * 128 : (c + 1) * 128],
                        lhsT=tm2s[c][:, m * 128 : (m + 1) * 128],
                        rhs=idt[:],
                        start=True,
                        stop=True,
                    )
                ym = spool.tile([128, ST], DTB, tag=f"ypa{m}")
                if m % 2 == 0:
                    nc.vector.tensor_copy(ym[:], ps[:])
                else:
                    nc.scalar.copy(ym[:], ps[:])
                ypa.append(ym)

            # ---- P5: moving FHT_h -> [(o',g), tok] f32; drain with beta scale
            for m in range(32):
                ps = pspool.tile([128, ST], DTF, tag="ps")
                nc.tensor.matmul(ps[:], lhsT=hmt[:], rhs=ypa[m][:], start=True, stop=True)
                yf = ypool.tile([128, ST], DTF, tag="yf")
                nc.scalar.activation(
                    yf[:],
                    ps[:],
                    mybir.ActivationFunctionType.Copy,
                    scale=bct[:, m : m + 1],
                )
                nc.sync.dma_start(yout[m * 128 : (m + 1) * 128, t0 : t0 + ST], yf[:])

        if loop_r == 1:
            for st in range(NST):
                supertile(st)
        else:
            with tc.For_i(0, loop_r, 1):
                for st in range(NST):
                    supertile(st)


@functools.lru_cache(maxsize=4)
def build_program(loop_r=1):
    nc = bacc.Bacc("TRN2", target_bir_lowering=False, debug=False)
    xin = nc.dram_tensor("xin", [TOKC, D], DTB, kind="ExternalInput").ap()
    hm = nc.dram_tensor("hmat", [128, 128], DTB, kind="ExternalInput").ap()
    idm = nc.dram_tensor("ident", [128, 128], DTB, kind="ExternalInput").ap()
    wqm = nc.dram_tensor("wqm", [128, G * OO], DTB, kind="ExternalInput").ap()
    bc = nc.dram_tensor("betacol", [128, G], DTF, kind="ExternalInput").ap()
    yout = nc.dram_tensor("yout", [D, TOKC], DTF, kind="ExternalOutput").ap()
    with tile.TileContext(nc) as tc:
        build_body(nc, tc, xin, hm, idm, wqm, bc, yout, loop_r=loop_r)
    nc.compile()
    return nc


def host_prep(x, weight, alpha, beta):
    """Returns (in_maps, decode_info). Pure f32 numpy glue + bf16 casts."""
    H = _hadamard(G)  # [g,h] +-1

    w = np.asarray(weight, dtype=np.float32)
    scale = np.float32(np.mean(np.abs(w))) + np.float32(1e-8)
    wq3 = np.clip(np.round(w / scale), -1.0, 1.0).astype(np.float32)  # [h,o,i] in {-1,0,1}

    # x * alpha, reorder features to i-major (f' = i*32+g)
    xp = np.asarray(x, dtype=np.float32).reshape(BT, G, IO) * np.asarray(
        alpha, dtype=np.float32
    )[None]
    xp = np.ascontiguousarray(xp.transpose(0, 2, 1)).reshape(BT, D)
    xin_all = xp.astype(BF16)

    hmat = np.kron(np.eye(4, dtype=np.float32), H).astype(BF16)  # [(i',g),(i'',h)]
    ident = np.eye(128, dtype=np.float32).astype(BF16)
    wq_sb = np.ascontiguousarray(wq3.transpose(2, 0, 1)).reshape(IO, G * OO).astype(BF16)  # [i,(h,o)]

    beta_f = np.asarray(beta, dtype=np.float32) * (scale / np.float32(G))  # [g,o]
    # betacol[p = o'*32+g, m] = beta_f[g, 4m+o']
    bc = np.ascontiguousarray(
        beta_f.T.reshape(G, 4, G).transpose(1, 2, 0)
    ).reshape(128, G).astype(np.float32)

    in_maps = []
    for c in range(NCORES):
        in_maps.append(
            {
                "xin": xin_all[c * TOKC : (c + 1) * TOKC],
                "hmat": hmat,
                "ident": ident,
                "wqm": wq_sb,
                "betacol": bc,
            }
        )
    return in_maps


def host_post(results):
    ydev = np.stack([r["yout"] for r in results])  # [8, 4096, 1024] f32
    # row r = m*128 + o'*32 + g  ->  feature (g, o = 4m+o'); want y[tok, g*128+o]
    y = ydev.reshape(NCORES, G, 4, G, TOKC)  # [c, m, o', g, tok]
    y = y.transpose(0, 4, 3, 1, 2)  # [c, tok, g, m, o']
    y = np.ascontiguousarray(y).reshape(BT, D)
    return y.reshape(B, T, D)


def kernel(x, weight, alpha, beta):
    nc = build_program(loop_r=1)
    in_maps = host_prep(x, weight, alpha, beta)
    res = run_bass_kernel_spmd(nc, in_maps, core_ids=list(range(NCORES)))
    return host_post(res.results)



# revision 15
# speedup vs baseline: 112.4107x; 112.4107x over previous
"""Trainium2 Bass kernel for HadamardTernaryLinear.

y = reshape( (FHT_g(x*alpha) @grouped w_q) -> FHT_h -> *beta ), with
w_q = BitNet-style absmean ternary quantization of weight.

Strategy: data-parallel over the 8192 tokens across 8 NeuronCores (1024
tokens/core, no collectives). Per core, a 3-pass TensorEngine pipeline in
bf16 (Hadamard and ternary weights are exact in bf16; quant scale, alpha
and beta fold into f32 host prep / the final drain):

  P1 FHT_g : MM(lhsT=I4(x)H32, rhs=xa_k)   32x N=512  -> xm [(i',h), tok]
  R1       : SBUF->SBUF regroup DMA        -> xb_h [i, tok] (h-grouped)
  P2 GM    : MM(lhsT=WqT_h,    rhs=xb_h)   32x N=512  -> yp_h [(o',m), tok]
  R2       : same regroup DMA pattern      -> ya_m [(h,o'), tok]
  P3 FHT_h : MM(lhsT=H32(x)I4, rhs=ya_m)   32x N=512  -> [(g,o'), tok]
  drain P3 on ScalarE with per-row scale beta*quantscale/32, bf16 out.

Both regroups are the same partition permutation dst[4c+a, b*512+t] =
src[a*32+b, c*512+t]: R2 reuses it by permuting the weight columns
host-side so P2's output partition index is o'*32+m (o = 4m+o').

PSUM->SBUF drains for P1/P2 rotate across Vector/GpSimd/Scalar engines.
Input arrives pre-transposed from host (feature-major, supertile-blocked)
so the input/output DMAs are fully linear (128 descriptors of 32KB).
"""

import functools
import sys

for _p in ("/opt/trn_rl_repo",):
    if _p not in sys.path:
        sys.path.insert(0, _p)

import ml_dtypes
import numpy as np

import concourse.mybir as mybir
import concourse.tile as tile
from concourse import bacc
from concourse.bass_utils import run_bass_kernel_spmd

G = 32
IO = 128  # in_o
OO = 128  # out_o
D = G * IO  # 4096
NCORES = 8
B, T = 4, 2048
BT = B * T
TOKC = BT // NCORES  # tokens per core
ST = 512  # supertile tokens
NST = TOKC // ST
KT = D // 128  # 32 feature tiles

DTB = mybir.dt.bfloat16
DTF = mybir.dt.float32
BF16 = ml_dtypes.bfloat16


def _hadamard(n):
    H = np.array([[1.0]], dtype=np.float32)
    while H.shape[0] < n:
        H = np.block([[H, H], [H, -H]])
    return H  # +-1, symmetric


def build_body(nc, tc, xin, hm, wqm, bc, yout, loop_r=1):
    """Emit the per-core program. All APs are DRAM tensors."""

    with (
        tc.tile_pool(name="const", bufs=1) as cpool,
        tc.tile_pool(name="xa", bufs=2) as xapool,
        tc.tile_pool(name="mid", bufs=1) as mpool,
        tc.tile_pool(name="ps1", bufs=3, space="PSUM") as ps1pool,
        tc.tile_pool(name="ps2", bufs=3, space="PSUM") as ps2pool,
        tc.tile_pool(name="ps3", bufs=2, space="PSUM") as ps3pool,
    ):
        hmt = cpool.tile([128, 128], DTB, tag="hm")
        nc.sync.dma_start(hmt[:], hm[0])
        hmt2 = cpool.tile([128, 128], DTB, tag="hm2")
        nc.sync.dma_start(hmt2[:], hm[1])
        wqt = cpool.tile([128, G * OO], DTB, tag="wq")
        nc.sync.dma_start(wqt[:], wqm[:])
        bct = cpool.tile([128, G], DTF, tag="bc")
        nc.sync.dma_start(bct[:], bc[:])

        # GPSIMD cannot read PSUM on trn2 — drains go Vector/Scalar only.
        # Vector uses tensor_scalar_mul for half the tiles as an A/B probe
        # (docs suggest tensor_scalar may hit a faster DVE perf mode).
        def drain(idx, o, i):
            if idx % 4 == 3:
                nc.scalar.copy(o, i)
            elif idx % 2 == 0:
                nc.vector.tensor_scalar_mul(o, i, 1.0)
            else:
                nc.vector.tensor_copy(o, i)

        def regroup(dst, src):
            # dst[4k+i', h*512+t] = src[i'*32+h, k*512+t].  Per k the source
            # walk (i', h, t) visits partitions 0..127 in order, so src is a
            # plain [128, 512] slice and dst is 4 contiguous partitions with
            # a fully contiguous 32KB free dim. Dispatch alternates SP/ACT.
            for k in range(KT):
                s = src[:, k * ST : (k + 1) * ST]
                d = dst[4 * k : 4 * k + 4, :].rearrange("i (h t) -> i h t", h=KT)
                eng = nc.scalar if k % 3 == 0 else nc.sync
                eng.dma_start(d, s)

        def supertile(st):
            # ---- load: fully linear (host pre-transposed, supertile-blocked)
            xa = xapool.tile([128, KT * ST], DTB, tag="xa")
            nc.sync.dma_start(xa[:], xin[st])

            # ---- P1: FHT_g, moving x -> xm_k [(i',h), tok] per i-block k
            xm = mpool.tile([128, KT * ST], DTB, tag="m1")
            for k in range(KT):
                ps = ps1pool.tile([128, ST], DTF, tag="ps1")
                nc.tensor.matmul(
                    ps[:], lhsT=hmt[:], rhs=xa[:, k * ST : (k + 1) * ST],
                    start=True, stop=True,
                )
                drain(k, xm[:, k * ST : (k + 1) * ST], ps[:])

            # ---- R1: regroup -> xb_h [i, tok] (h-grouped)
            xb = mpool.tile([128, KT * ST], DTB, tag="m2")
            regroup(xb, xm)

            # ---- P2: grouped ternary matmul -> yp_h [o, tok]
            yp = mpool.tile([128, KT * ST], DTB, tag="m3")
            for h in range(KT):
                ps = ps2pool.tile([128, ST], DTF, tag="ps2")
                nc.tensor.matmul(
                    ps[:], lhsT=wqt[:, h * 128 : (h + 1) * 128],
                    rhs=xb[:, h * ST : (h + 1) * ST],
                    start=True, stop=True,
                )
                drain(h + 1, yp[:, h * ST : (h + 1) * ST], ps[:])

            # ---- R2: regroup -> ya_m [(o',h), tok] per o-block m
            ya = mpool.tile([128, KT * ST], DTB, tag="m1")
            regroup(ya, yp)

            # ---- P3: FHT_h -> [(g,o'), tok]; drain with beta scale, bf16
            yf = mpool.tile([128, KT * ST], DTB, tag="m2")
            for m in range(KT):
                ps = ps3pool.tile([128, ST], DTF, tag="ps3")
                nc.tensor.matmul(
                    ps[:], lhsT=hmt2[:], rhs=ya[:, m * ST : (m + 1) * ST],
                    start=True, stop=True,
                )
                if m % 2 == 0:
                    nc.scalar.activation(
                        yf[:, m * ST : (m + 1) * ST], ps[:],
                        mybir.ActivationFunctionType.Copy,
                        scale=bct[:, m : m + 1],
                    )
                else:
                    nc.vector.tensor_scalar_mul(
                        yf[:, m * ST : (m + 1) * ST], ps[:], bct[:, m : m + 1]
                    )
            nc.sync.dma_start(yout[st], yf[:])

        if loop_r == 1:
            for st in range(NST):
                supertile(st)
        else:
            with tc.For_i(0, loop_r, 1):
                for st in range(NST):
                    supertile(st)


@functools.lru_cache(maxsize=4)
def build_program(loop_r=1):
    nc = bacc.Bacc("TRN2", target_bir_lowering=False, debug=False)
    xin = nc.dram_tensor("xin", [NST, 128, KT * ST], DTB, kind="ExternalInput").ap()
    hm = nc.dram_tensor("hmat", [2, 128, 128], DTB, kind="ExternalInput").ap()
    wqm = nc.dram_tensor("wqm", [128, G * OO], DTB, kind="ExternalInput").ap()
    bc = nc.dram_tensor("betacol", [128, G], DTF, kind="ExternalInput").ap()
    yout = nc.dram_tensor("yout", [NST, 128, KT * ST], DTB, kind="ExternalOutput").ap()
    with tile.TileContext(nc) as tc:
        build_body(nc, tc, xin, hm, wqm, bc, yout, loop_r=loop_r)
    nc.compile()
    return nc


def host_prep(x, weight, alpha, beta):
    """Returns per-core input maps. Pure f32 numpy glue + bf16 casts."""
    H = _hadamard(G)  # [g,h] +-1

    w = np.asarray(weight, dtype=np.float32)
    scale = np.float32(np.mean(np.abs(w))) + np.float32(1e-8)
    wq3 = np.clip(np.round(w / scale), -1.0, 1.0).astype(np.float32)  # [h,o,i]

    # x * alpha, feature order f' = i*32+g (i-major)
    xp = np.asarray(x, dtype=np.float32).reshape(BT, G, IO) * np.asarray(
        alpha, dtype=np.float32
    )[None]
    xp = np.ascontiguousarray(xp.transpose(0, 2, 1)).reshape(BT, D)  # [tok, f']
    # device layout: [core, st, p(128), k(32), t(512)] with f' = k*128 + p
    xin_all = np.ascontiguousarray(
        xp.reshape(NCORES, NST, ST, KT, 128).transpose(0, 1, 4, 3, 2)
    ).reshape(NCORES, NST, 128, KT * ST).astype(BF16)

    hmat = np.stack(
        [
            np.kron(np.eye(4, dtype=np.float32), H),  # P1: I4 (x) H
            np.kron(H, np.eye(4, dtype=np.float32)),  # P3: H (x) I4
        ]
    ).astype(BF16)
    # wq_sb[i, h*128 + o'*32 + m] = wq3[h, 4m+o', i]  (o = 4m+o')
    wq_sb = np.ascontiguousarray(
        wq3.reshape(G, G, 4, IO).transpose(3, 0, 2, 1)
    ).reshape(IO, G * OO).astype(BF16)

    beta_f = np.asarray(beta, dtype=np.float32) * (scale / np.float32(G))  # [g,o]
    # betacol[p = 4g+o', m] = beta_f[g, 4m+o']
    bct = np.ascontiguousarray(
        beta_f.reshape(G, G, 4).transpose(0, 2, 1)
    ).reshape(128, G).astype(np.float32)

    in_maps = []
    for c in range(NCORES):
        in_maps.append(
            {
                "xin": xin_all[c],
                "hmat": hmat,
                "wqm": wq_sb,
                "betacol": bct,
            }
        )
    return in_maps


def host_post(results):
    ydev = np.stack([r["yout"] for r in results])  # [8, NST, 128, 16384] bf16
    # [c, st, p=(4g+o'), m*512+t] -> y[tok, g*128+4m+o']
    y = ydev.reshape(NCORES, NST, G, 4, KT, ST)  # [c, st, g, o', m, t]
    y = y.transpose(0, 1, 5, 2, 4, 3)  # [c, st, t, g, m, o']
    y = np.ascontiguousarray(y).astype(np.float32).reshape(BT, D)
    return y.reshape(B, T, D)


def kernel(x, weight, alpha, beta):
    nc = build_program(loop_r=1)
    in_maps = host_prep(x, weight, alpha, beta)
    res = run_bass_kernel_spmd(nc, in_maps, core_ids=list(range(NCORES)))
    return host_post(res.results)
